# revision 6
# baseline (speedup 1.0000x reference)
"""Trainium2 Bass kernel: BiDAF-style context-query attention (nn_CQattn).

Reference (per batch b):
    S    = (C@w1)[:,None] + (Q@w2)[None,:] + (C*w3) @ Q.T        # [N, M]
    S1   = softmax_m(S + NEG*Qmask[None,:])                      # row softmax
    S2   = softmax_n(S + NEG*Cmask[:,None])                      # col softmax
    A    = S1 @ Q                                                # [N, D]
    Bout = S1 @ (S2.T @ C)                                       # [N, D]

Key algebra used on device:
  - softmax_m(S + c1[n] + ...) drops the per-row c1 term (constant in m);
    softmax_n drops the per-col q2 term.  So only one additive bias per
    softmax survives, and it is per-PSUM-partition in the right layout:
      E2  = exp(dot3[n,m]  + c1m[n])   (natural layout, bias per partition)
      E1T = exp(dot3T[m,n] + q2m[m])   (transposed layout, bias per partition)
    where dot3 = (C) @ diag(w3) @ Q.T, c1m = C@w1 + NEG*Cmask,
    q2m = Q@w2 + NEG*Qmask.  Max-subtraction is skipped: |S| <= ~10 for
    this data, exp() stays well inside fp32 range, and masked entries
    round to exactly -1e30 (|S| << ulp(1e30)) so exp -> 0 exactly.
  - Row/col sums of E1T/E2 are computed on the PE with a ones[128,1] rhs
    sharing the stationary operand with the big matmuls.
  - A = diag(1/rowsum1) @ (E1T.T @ Q), Bout = diag(1/rowsum1) @ (E1T.T @ T),
    T = diag(1/colsum2) @ (E2.T-contracted vs C); the diagonal scalings are
    per-partition scales applied on PSUM->SBUF eviction (ACT Copy w/ scale).

Sharding: data-parallel over batch: 32 batches / 8 cores = 4 per core.
Self-contained: shapes hardcoded; no sibling imports.
"""

import os
import numpy as np

B, N, M, D = 32, 2048, 512, 512
NCORES = 8
BPC = B // NCORES  # batches per core
NEG = -1e30

NT = N // 128  # 16 n-tiles
MT = M // 128  # 4 m-tiles
DT = D // 128  # 4 d-tiles
NQ = N // 512  # 4 groups of 4 n-tiles


def _patch_tile_drain_wait_split():
    """The stock Tile kernel-tail drain carries one sem-wait per still-pending
    proc on a single InstDrain; the walrus build in this container rejects >1
    sync wait per instruction ("Too many sync wait commands").  Split the
    excess waits onto dedicated sync-engine NOPs emitted right after the
    drain (they still precede the all-engine barrier, preserving the
    everything-done-before-teardown guarantee)."""
    import concourse.mybir as mybir
    import concourse.tile as tile

    if getattr(tile.TileContext, "_drain_wait_split_patched", False):
        return

    orig_add = tile.TileContext._add_instruction

    def _add_instruction(self, inst):
        si = inst.sync_info
        waits = list(si.on_wait) if si and si.on_wait else []
        if len(waits) > 1 and inst.engine != mybir.EngineType.Unassigned:
            for w in waits[:-1]:
                nop = mybir.InstNoOp(
                    name=self.nc.get_next_instruction_name(), ins=[], outs=[]
                )
                nop.engine = inst.engine
                nop.sync_info = mybir.SyncInfo(on_wait=[w], on_update=[])
                orig_add(self, nop)
            inst.sync_info = mybir.SyncInfo(
                on_wait=[waits[-1]],
                on_update=list(si.on_update) if si.on_update else [],
            )
        orig_add(self, inst)

    tile.TileContext._add_instruction = _add_instruction

    def _drain_and_barrier(self, tick_clock, wait_clock):
        nc = self.nc
        drain_inst = nc.sync.drain()
        wait_clock.add_sem_waits(
            drain_inst.ins, tile.ScopedClock({None: tick_clock.global_clock})
        )
        si = drain_inst.ins.sync_info
        waits = list(si.on_wait) if si and si.on_wait else []
        if len(waits) > 1:
            drain_inst.ins.sync_info = mybir.SyncInfo(
                on_wait=[waits[0]],
                on_update=list(si.on_update) if si and si.on_update else [],
            )
            for w in waits[1:]:
                nop = nc.sync.nop(nofuse=True, hint="drain_wait_split")
                nop.ins.sync_info = mybir.SyncInfo(on_wait=[w], on_update=[])

        nc.all_engine_barrier()
        assert self.sems is not None
        popped = nc._tile_sem_poison_stack.pop()
        assert popped is self._sem_poison
        nc.clear_and_free_semaphores(list(self.sems.allocated().values()))
        nc.all_engine_barrier()

    tile.TileContext._drain_and_barrier = _drain_and_barrier
    tile.TileContext._drain_wait_split_patched = True


def build_nc():
    import concourse.bass as bass
    import concourse.mybir as mybir
    import concourse.tile as tile

    _patch_tile_drain_wait_split()

    f32 = mybir.dt.float32
    AF = mybir.ActivationFunctionType

    nc = bass.Bass()
    C_d = nc.dram_tensor("C", [BPC, N, D], f32, kind="ExternalInput")
    Q_d = nc.dram_tensor("Q", [BPC, M, D], f32, kind="ExternalInput")
    cmb_d = nc.dram_tensor("cmb", [128, BPC, NT], f32, kind="ExternalInput")
    qmb_d = nc.dram_tensor("qmb", [128, BPC, MT], f32, kind="ExternalInput")
    w1_d = nc.dram_tensor("w1r", [128, DT], f32, kind="ExternalInput")
    w2_d = nc.dram_tensor("w2r", [128, DT], f32, kind="ExternalInput")
    w3_d = nc.dram_tensor("w3r", [128, DT], f32, kind="ExternalInput")
    id_d = nc.dram_tensor("ident", [128, 128], f32, kind="ExternalInput")
    A_d = nc.dram_tensor("A", [BPC, N, D], f32, kind="ExternalOutput")
    Bo_d = nc.dram_tensor("Bout", [BPC, N, D], f32, kind="ExternalOutput")

    with tile.TileContext(nc) as tc:
        with (
            tc.tile_pool(name="const", bufs=1) as constp,
            tc.tile_pool(name="cin", bufs=4) as cpool,
            tc.tile_pool(name="qin", bufs=2) as qpool,
            tc.tile_pool(name="ctp", bufs=4) as ctpool,
            tc.tile_pool(name="qtp", bufs=4) as qtpool,
            tc.tile_pool(name="qwtp", bufs=4) as qwtpool,
            tc.tile_pool(name="e2p", bufs=16) as e2pool,
            tc.tile_pool(name="e1tp", bufs=4) as e1tpool,
            tc.tile_pool(name="tp", bufs=4) as tpool,
            tc.tile_pool(name="smallp", bufs=24) as smallpool,
            tc.tile_pool(name="stagep", bufs=2) as stagepool,
            tc.tile_pool(name="psbig", bufs=4, space="PSUM") as psb,
            tc.tile_pool(name="pssmall", bufs=3, space="PSUM") as pss,
        ):
            ident = constp.tile([128, 128], f32, name="ident")
            nc.sync.dma_start(ident[:], id_d[:])
            ones = constp.tile([128, 1], f32, name="ones")
            nc.vector.memset(ones[:], 1.0)
            w1r = constp.tile([128, DT], f32, name="w1r")
            nc.sync.dma_start(w1r[:], w1_d[:])
            w2r = constp.tile([128, DT], f32, name="w2r")
            nc.sync.dma_start(w2r[:], w2_d[:])
            w3r = constp.tile([128, DT], f32, name="w3r")
            nc.sync.dma_start(w3r[:], w3_d[:])
            cmb = constp.tile([128, BPC, NT], f32, name="cmb")
            nc.sync.dma_start(cmb[:], cmb_d[:])
            qmb = constp.tile([128, BPC, MT], f32, name="qmb")
            nc.sync.dma_start(qmb[:], qmb_d[:])

            for b in range(BPC):
                # ---- load C (16 n-tiles in 4 sbuf tiles) and Q (4 m-tiles)
                c_tiles = []
                for q in range(NQ):
                    cin = cpool.tile([128, 4, D], f32, name="Cin", tag="Cin")
                    nc.sync.dma_start(
                        cin[:],
                        C_d[b, q * 512 : (q + 1) * 512, :].rearrange(
                            "(s p) d -> p s d", p=128
                        ),
                    )
                    c_tiles.append(cin)
                q_in = qpool.tile([128, MT, D], f32, name="Qin", tag="Qin")
                nc.sync.dma_start(
                    q_in[:], Q_d[b].rearrange("(s p) d -> p s d", p=128)
                )

                def Cn(t):
                    return c_tiles[t // 4][:, t % 4, :]

                def Qm(u):
                    return q_in[:, u, :]

                # ---- transpose C -> CT[j] = [128 d, 2048 n] via PE (identity rhs)
                ctd = [
                    ctpool.tile([128, N], f32, name=f"CT{j}", tag="CT")
                    for j in range(DT)
                ]
                for tq in range(NQ):
                    for j in range(DT):
                        ps = psb.tile([128, 512], f32, name="ps_tr", tag="psb")
                        for s in range(4):
                            t = tq * 4 + s
                            nc.tensor.matmul(
                                ps[:, s * 128 : (s + 1) * 128],
                                Cn(t)[:, j * 128 : (j + 1) * 128],
                                ident[:],
                            )
                        nc.vector.tensor_copy(
                            ctd[j][:, tq * 512 : (tq + 1) * 512], ps[:]
                        )

                # ---- transpose Q -> QT[j], QwT[j] = QT * w3 (per-partition d)
                qtd, qwtd = [], []
                for j in range(DT):
                    ps = psb.tile([128, 512], f32, name="ps_trq", tag="psb")
                    for u in range(MT):
                        nc.tensor.matmul(
                            ps[:, u * 128 : (u + 1) * 128],
                            Qm(u)[:, j * 128 : (j + 1) * 128],
                            ident[:],
                        )
                    qtj = qtpool.tile([128, M], f32, name=f"QT{j}", tag="QT")
                    nc.vector.tensor_copy(qtj[:], ps[:])
                    qwtj = qwtpool.tile([128, M], f32, name=f"QwT{j}", tag="QwT")
                    nc.vector.tensor_scalar_mul(qwtj[:], ps[:], w3r[:, j : j + 1])
                    qtd.append(qtj)
                    qwtd.append(qwtj)

                # ---- q2m[u] = QT.T @ w2 + NEG*Qmask  (per m-tile, [128,1])
                q2m_tiles = []
                for u in range(MT):
                    psq = pss.tile([128, 1], f32, name="ps_q2", tag="pss")
                    for j in range(DT):
                        nc.tensor.matmul(
                            psq[:],
                            qtd[j][:, u * 128 : (u + 1) * 128],
                            w2r[:, j : j + 1],
                            start=(j == 0),
                            stop=(j == DT - 1),
                        )
                    q2m_u = smallpool.tile([128, 1], f32, name="q2m", tag="small")
                    nc.vector.tensor_add(q2m_u[:], psq[:], qmb[:, b, u : u + 1])
                    q2m_tiles.append(q2m_u)

                # ---- E2[t] = exp(dot3 + c1m[t]) ; c1 fused on same lhsT
                e2_tiles = []
                for t in range(NT):
                    pse = psb.tile([128, 512], f32, name="ps_e2", tag="psb")
                    psc = pss.tile([128, 1], f32, name="ps_c1", tag="pss")
                    for j in range(DT):
                        lhsT = ctd[j][:, t * 128 : (t + 1) * 128]
                        nc.tensor.matmul(
                            pse[:], lhsT, qwtd[j][:],
                            start=(j == 0), stop=(j == DT - 1),
                        )
                        nc.tensor.matmul(
                            psc[:], lhsT, w1r[:, j : j + 1],
                            start=(j == 0), stop=(j == DT - 1),
                        )
                    c1m_t = smallpool.tile([128, 1], f32, name="c1m", tag="small")
                    nc.vector.tensor_add(c1m_t[:], psc[:], cmb[:, b, t : t + 1])
                    e2t = e2pool.tile([128, 512], f32, name="E2", tag="E2")
                    nc.scalar.activation(e2t[:], pse[:], AF.Exp, bias=c1m_t[:])
                    e2_tiles.append(e2t)

                # ---- E1T[u] = exp(dot3T + q2m[u])  [128 m, 2048 n]
                e1t_tiles = []
                for u in range(MT):
                    e1tu = e1tpool.tile([128, N], f32, name="E1T", tag="E1T")
                    ps4 = [
                        psb.tile([128, 512], f32, name=f"ps_e1_{k}", tag="psb")
                        for k in range(NQ)
                    ]
                    for j in range(DT):
                        lhsT = qwtd[j][:, u * 128 : (u + 1) * 128]
                        for nq in range(NQ):
                            nc.tensor.matmul(
                                ps4[nq][:],
                                lhsT,
                                ctd[j][:, nq * 512 : (nq + 1) * 512],
                                start=(j == 0),
                                stop=(j == DT - 1),
                            )
                    for nq in range(NQ):
                        nc.scalar.activation(
                            e1tu[:, nq * 512 : (nq + 1) * 512],
                            ps4[nq][:],
                            AF.Exp,
                            bias=q2m_tiles[u][:],
                        )
                    e1t_tiles.append(e1tu)

                # ---- T[u] = (1/colsum2) * sum_n E2[n, m-tile u] * C[n, :]
                t_tiles = []
                for u in range(MT):
                    pst = psb.tile([128, 512], f32, name="ps_T", tag="psb")
                    psc = pss.tile([128, 1], f32, name="ps_cs", tag="pss")
                    for t in range(NT):
                        lhsT = e2_tiles[t][:, u * 128 : (u + 1) * 128]
                        nc.tensor.matmul(
                            pst[:], lhsT, Cn(t)[:],
                            start=(t == 0), stop=(t == NT - 1),
                        )
                        nc.tensor.matmul(
                            psc[:], lhsT, ones[:],
                            start=(t == 0), stop=(t == NT - 1),
                        )
                    r2u = smallpool.tile([128, 1], f32, name="r2", tag="small")
                    nc.vector.reciprocal(r2u[:], psc[:])
                    ttu = tpool.tile([128, 512], f32, name="T", tag="T")
                    nc.scalar.activation(ttu[:], pst[:], AF.Copy, scale=r2u[:])
                    t_tiles.append(ttu)

                # ---- A[t] / Bout[t] = (1/rowsum1) * E1T.T @ {Q, T}
                for g in range(NT // 2):
                    ast = stagepool.tile([128, 2, D], f32, name="Ast", tag="Ast")
                    bst = stagepool.tile([128, 2, D], f32, name="Bst", tag="Bst")
                    for s in range(2):
                        t = g * 2 + s
                        psa = psb.tile([128, 512], f32, name="ps_A", tag="psb")
                        psbb = psb.tile([128, 512], f32, name="ps_B", tag="psb")
                        psr = pss.tile([128, 1], f32, name="ps_rs", tag="pss")
                        for u in range(MT):
                            lhsT = e1t_tiles[u][:, t * 128 : (t + 1) * 128]
                            nc.tensor.matmul(
                                psa[:], lhsT, Qm(u)[:],
                                start=(u == 0), stop=(u == MT - 1),
                            )
                            nc.tensor.matmul(
                                psbb[:], lhsT, t_tiles[u][:],
                                start=(u == 0), stop=(u == MT - 1),
                            )
                            nc.tensor.matmul(
                                psr[:], lhsT, ones[:],
                                start=(u == 0), stop=(u == MT - 1),
                            )
                        r1t = smallpool.tile([128, 1], f32, name="r1", tag="small")
                        nc.vector.reciprocal(r1t[:], psr[:])
                        nc.scalar.activation(
                            ast[:, s, :], psa[:], AF.Copy, scale=r1t[:]
                        )
                        nc.scalar.activation(
                            bst[:, s, :], psbb[:], AF.Copy, scale=r1t[:]
                        )
                    nc.sync.dma_start(
                        A_d[b, g * 256 : (g + 1) * 256, :].rearrange(
                            "(s p) d -> p s d", p=128
                        ),
                        ast[:],
                    )
                    nc.sync.dma_start(
                        Bo_d[b, g * 256 : (g + 1) * 256, :].rearrange(
                            "(s p) d -> p s d", p=128
                        ),
                        bst[:],
                    )

    return nc


_NC = None


def _get_nc():
    global _NC
    if _NC is None:
        _NC = build_nc()
        _NC.finalize()
    return _NC


def _make_in_maps(C, Q, Cmask, Qmask, w):
    C = np.asarray(C, dtype=np.float32)
    Q = np.asarray(Q, dtype=np.float32)
    w = np.asarray(w, dtype=np.float32)
    w1, w2, w3 = w[:D], w[D : 2 * D], w[2 * D :]
    w1r = np.ascontiguousarray(w1.reshape(DT, 128).T)
    w2r = np.ascontiguousarray(w2.reshape(DT, 128).T)
    w3r = np.ascontiguousarray(w3.reshape(DT, 128).T)
    ident = np.eye(128, dtype=np.float32)
    cmb_full = np.asarray(Cmask, dtype=np.float32) * np.float32(NEG)  # [B, N]
    qmb_full = np.asarray(Qmask, dtype=np.float32) * np.float32(NEG)  # [B, M]

    in_maps = []
    for c in range(NCORES):
        bs = slice(c * BPC, (c + 1) * BPC)
        cmb = np.ascontiguousarray(
            cmb_full[bs].reshape(BPC, NT, 128).transpose(2, 0, 1)
        )
        qmb = np.ascontiguousarray(
            qmb_full[bs].reshape(BPC, MT, 128).transpose(2, 0, 1)
        )
        in_maps.append(
            {
                "C": np.ascontiguousarray(C[bs]),
                "Q": np.ascontiguousarray(Q[bs]),
                "cmb": cmb,
                "qmb": qmb,
                "w1r": w1r,
                "w2r": w2r,
                "w3r": w3r,
                "ident": ident,
            }
        )
    return in_maps


def run_spmd(C, Q, Cmask, Qmask, w, trace=False):
    """Returns ((A, Bout), BassKernelResults)."""
    from concourse.bass_utils import run_bass_kernel_spmd

    nc = _get_nc()
    in_maps = _make_in_maps(C, Q, Cmask, Qmask, w)
    res = run_bass_kernel_spmd(nc, in_maps, list(range(NCORES)), trace=trace)
    A = np.concatenate([np.asarray(r["A"]) for r in res.results], axis=0)
    Bout = np.concatenate([np.asarray(r["Bout"]) for r in res.results], axis=0)
    return (A, Bout), res


def kernel(C, Q, Cmask, Qmask, w):
    (A, Bout), _ = run_spmd(
        C, Q, Cmask, Qmask, w, trace=bool(int(os.environ.get("BASSK_TRACE", "0")))
    )
    return (A, Bout)


# revision 8
# speedup vs baseline: 8655.9791x; 8655.9791x over previous
"""Trainium2 Bass kernel: BiDAF-style context-query attention (nn_CQattn).

Reference (per batch b):
    S    = (C@w1)[:,None] + (Q@w2)[None,:] + (C*w3) @ Q.T        # [N, M]
    S1   = softmax_m(S + NEG*Qmask[None,:])                      # row softmax
    S2   = softmax_n(S + NEG*Cmask[:,None])                      # col softmax
    A    = S1 @ Q                                                # [N, D]
    Bout = S1 @ (S2.T @ C)                                       # [N, D]

Key algebra used on device:
  - softmax_m(S + c1[n] + ...) drops the per-row c1 term (constant in m);
    softmax_n drops the per-col q2 term.  So only one additive bias per
    softmax survives, and it is per-PSUM-partition in the right layout:
      E2  = exp(dot3[n,m]  + c1m[n])   (natural layout, bias per partition)
      E1T = exp(dot3T[m,n] + q2m[m])   (transposed layout, bias per partition)
    where dot3 = (C) @ diag(w3) @ Q.T, c1m = C@w1 + NEG*Cmask,
    q2m = Q@w2 + NEG*Qmask.  Max-subtraction is skipped: |S| <= ~10 for
    this data, exp() stays well inside fp32 range, and masked entries
    round to exactly -1e30 (|S| << ulp(1e30)) so exp -> 0 exactly.
  - Row/col sums of E1T/E2 are computed on the PE with a ones[128,1] rhs
    sharing the stationary operand with the big matmuls.
  - A = diag(1/rowsum1) @ (E1T.T @ Q), Bout = diag(1/rowsum1) @ (E1T.T @ T),
    T = diag(1/colsum2) @ (E2.T-contracted vs C); the diagonal scalings are
    per-partition scales applied on PSUM->SBUF eviction (ACT Copy w/ scale).

Sharding: data-parallel over batch: 32 batches / 8 cores = 4 per core.
Self-contained: shapes hardcoded; no sibling imports.
"""

import os
import numpy as np

B, N, M, D = 32, 2048, 512, 512
NCORES = 8
BPC = B // NCORES  # batches per core
NEG = -1e30

NT = N // 128  # 16 n-tiles
MT = M // 128  # 4 m-tiles
DT = D // 128  # 4 d-tiles
NQ = N // 512  # 4 groups of 4 n-tiles


def _patch_tile_drain_wait_split():
    """The stock Tile kernel-tail drain carries one sem-wait per still-pending
    proc on a single InstDrain; the walrus build in this container rejects >1
    sync wait per instruction ("Too many sync wait commands").  Split the
    excess waits onto dedicated sync-engine NOPs emitted right after the
    drain (they still precede the all-engine barrier, preserving the
    everything-done-before-teardown guarantee)."""
    import concourse.mybir as mybir
    import concourse.tile as tile

    if getattr(tile.TileContext, "_drain_wait_split_patched", False):
        return

    orig_add = tile.TileContext._add_instruction

    def _add_instruction(self, inst):
        si = inst.sync_info
        waits = list(si.on_wait) if si and si.on_wait else []
        if len(waits) > 1 and inst.engine != mybir.EngineType.Unassigned:
            for w in waits[:-1]:
                nop = mybir.InstNoOp(
                    name=self.nc.get_next_instruction_name(), ins=[], outs=[]
                )
                nop.engine = inst.engine
                nop.sync_info = mybir.SyncInfo(on_wait=[w], on_update=[])
                orig_add(self, nop)
            inst.sync_info = mybir.SyncInfo(
                on_wait=[waits[-1]],
                on_update=list(si.on_update) if si.on_update else [],
            )
        orig_add(self, inst)

    tile.TileContext._add_instruction = _add_instruction

    def _drain_and_barrier(self, tick_clock, wait_clock):
        nc = self.nc
        drain_inst = nc.sync.drain()
        wait_clock.add_sem_waits(
            drain_inst.ins, tile.ScopedClock({None: tick_clock.global_clock})
        )
        si = drain_inst.ins.sync_info
        waits = list(si.on_wait) if si and si.on_wait else []
        if len(waits) > 1:
            drain_inst.ins.sync_info = mybir.SyncInfo(
                on_wait=[waits[0]],
                on_update=list(si.on_update) if si and si.on_update else [],
            )
            for w in waits[1:]:
                nop = nc.sync.nop(nofuse=True, hint="drain_wait_split")
                nop.ins.sync_info = mybir.SyncInfo(on_wait=[w], on_update=[])

        nc.all_engine_barrier()
        assert self.sems is not None
        popped = nc._tile_sem_poison_stack.pop()
        assert popped is self._sem_poison
        nc.clear_and_free_semaphores(list(self.sems.allocated().values()))
        nc.all_engine_barrier()

    tile.TileContext._drain_and_barrier = _drain_and_barrier
    tile.TileContext._drain_wait_split_patched = True


def build_nc(n_reps=1):
    import concourse.bass as bass
    import concourse.mybir as mybir
    import concourse.tile as tile

    _patch_tile_drain_wait_split()

    f32 = mybir.dt.float32
    AF = mybir.ActivationFunctionType

    nc = bass.Bass()
    C_d = nc.dram_tensor("C", [BPC, N, D], f32, kind="ExternalInput")
    Q_d = nc.dram_tensor("Q", [BPC, M, D], f32, kind="ExternalInput")
    cmb_d = nc.dram_tensor("cmb", [128, BPC, NT], f32, kind="ExternalInput")
    qmb_d = nc.dram_tensor("qmb", [128, BPC, MT], f32, kind="ExternalInput")
    w1_d = nc.dram_tensor("w1r", [128, DT], f32, kind="ExternalInput")
    w2_d = nc.dram_tensor("w2r", [128, DT], f32, kind="ExternalInput")
    w3_d = nc.dram_tensor("w3r", [128, DT], f32, kind="ExternalInput")
    id_d = nc.dram_tensor("ident", [128, 128], f32, kind="ExternalInput")
    A_d = nc.dram_tensor("A", [BPC, N, D], f32, kind="ExternalOutput")
    Bo_d = nc.dram_tensor("Bout", [BPC, N, D], f32, kind="ExternalOutput")

    with tile.TileContext(nc) as tc:
        with (
            tc.tile_pool(name="const", bufs=1) as constp,
            tc.tile_pool(name="cin", bufs=4) as cpool,
            tc.tile_pool(name="qin", bufs=2) as qpool,
            tc.tile_pool(name="ctp", bufs=4) as ctpool,
            tc.tile_pool(name="qtp", bufs=4) as qtpool,
            tc.tile_pool(name="qwtp", bufs=4) as qwtpool,
            tc.tile_pool(name="e2p", bufs=16) as e2pool,
            tc.tile_pool(name="e1tp", bufs=4) as e1tpool,
            tc.tile_pool(name="tp", bufs=4) as tpool,
            tc.tile_pool(name="smallp", bufs=24) as smallpool,
            tc.tile_pool(name="stagep", bufs=2) as stagepool,
            tc.tile_pool(name="psbig", bufs=4, space="PSUM") as psb,
            tc.tile_pool(name="pssmall", bufs=3, space="PSUM") as pss,
        ):
            ident = constp.tile([128, 128], f32, name="ident")
            nc.sync.dma_start(ident[:], id_d[:])
            ones = constp.tile([128, 1], f32, name="ones")
            nc.vector.memset(ones[:], 1.0)
            w1r = constp.tile([128, DT], f32, name="w1r")
            nc.sync.dma_start(w1r[:], w1_d[:])
            w2r = constp.tile([128, DT], f32, name="w2r")
            nc.sync.dma_start(w2r[:], w2_d[:])
            w3r = constp.tile([128, DT], f32, name="w3r")
            nc.sync.dma_start(w3r[:], w3_d[:])
            cmb = constp.tile([128, BPC, NT], f32, name="cmb")
            nc.sync.dma_start(cmb[:], cmb_d[:])
            qmb = constp.tile([128, BPC, MT], f32, name="qmb")
            nc.sync.dma_start(qmb[:], qmb_d[:])

            for b in [b for _ in range(n_reps) for b in range(BPC)]:
                # ---- load C (16 n-tiles in 4 sbuf tiles) and Q (4 m-tiles)
                c_tiles = []
                for q in range(NQ):
                    cin = cpool.tile([128, 4, D], f32, name="Cin", tag="Cin")
                    nc.sync.dma_start(
                        cin[:],
                        C_d[b, q * 512 : (q + 1) * 512, :].rearrange(
                            "(s p) d -> p s d", p=128
                        ),
                    )
                    c_tiles.append(cin)
                q_in = qpool.tile([128, MT, D], f32, name="Qin", tag="Qin")
                nc.sync.dma_start(
                    q_in[:], Q_d[b].rearrange("(s p) d -> p s d", p=128)
                )

                def Cn(t):
                    return c_tiles[t // 4][:, t % 4, :]

                def Qm(u):
                    return q_in[:, u, :]

                # ---- transpose C -> CT[j] = [128 d, 2048 n] via PE (identity rhs)
                ctd = [
                    ctpool.tile([128, N], f32, name=f"CT{j}", tag="CT")
                    for j in range(DT)
                ]
                for tq in range(NQ):
                    for j in range(DT):
                        ps = psb.tile([128, 512], f32, name="ps_tr", tag="psb")
                        for s in range(4):
                            t = tq * 4 + s
                            nc.tensor.matmul(
                                ps[:, s * 128 : (s + 1) * 128],
                                Cn(t)[:, j * 128 : (j + 1) * 128],
                                ident[:],
                            )
                        nc.vector.tensor_copy(
                            ctd[j][:, tq * 512 : (tq + 1) * 512], ps[:]
                        )

                # ---- transpose Q -> QT[j], QwT[j] = QT * w3 (per-partition d)
                qtd, qwtd = [], []
                for j in range(DT):
                    ps = psb.tile([128, 512], f32, name="ps_trq", tag="psb")
                    for u in range(MT):
                        nc.tensor.matmul(
                            ps[:, u * 128 : (u + 1) * 128],
                            Qm(u)[:, j * 128 : (j + 1) * 128],
                            ident[:],
                        )
                    qtj = qtpool.tile([128, M], f32, name=f"QT{j}", tag="QT")
                    nc.vector.tensor_copy(qtj[:], ps[:])
                    qwtj = qwtpool.tile([128, M], f32, name=f"QwT{j}", tag="QwT")
                    nc.vector.tensor_scalar_mul(qwtj[:], ps[:], w3r[:, j : j + 1])
                    qtd.append(qtj)
                    qwtd.append(qwtj)

                # ---- q2m[u] = QT.T @ w2 + NEG*Qmask  (per m-tile, [128,1])
                q2m_tiles = []
                for u in range(MT):
                    psq = pss.tile([128, 1], f32, name="ps_q2", tag="pss")
                    for j in range(DT):
                        nc.tensor.matmul(
                            psq[:],
                            qtd[j][:, u * 128 : (u + 1) * 128],
                            w2r[:, j : j + 1],
                            start=(j == 0),
                            stop=(j == DT - 1),
                        )
                    q2m_u = smallpool.tile([128, 1], f32, name="q2m", tag="small")
                    nc.vector.tensor_add(q2m_u[:], psq[:], qmb[:, b, u : u + 1])
                    q2m_tiles.append(q2m_u)

                # ---- E2[t] = exp(dot3 + c1m[t]) ; c1 fused on same lhsT
                e2_tiles = []
                for t in range(NT):
                    pse = psb.tile([128, 512], f32, name="ps_e2", tag="psb")
                    psc = pss.tile([128, 1], f32, name="ps_c1", tag="pss")
                    for j in range(DT):
                        lhsT = ctd[j][:, t * 128 : (t + 1) * 128]
                        nc.tensor.matmul(
                            pse[:], lhsT, qwtd[j][:],
                            start=(j == 0), stop=(j == DT - 1),
                        )
                        nc.tensor.matmul(
                            psc[:], lhsT, w1r[:, j : j + 1],
                            start=(j == 0), stop=(j == DT - 1),
                        )
                    c1m_t = smallpool.tile([128, 1], f32, name="c1m", tag="small")
                    nc.vector.tensor_add(c1m_t[:], psc[:], cmb[:, b, t : t + 1])
                    e2t = e2pool.tile([128, 512], f32, name="E2", tag="E2")
                    nc.scalar.activation(e2t[:], pse[:], AF.Exp, bias=c1m_t[:])
                    e2_tiles.append(e2t)

                # ---- E1T[u] = exp(dot3T + q2m[u])  [128 m, 2048 n]
                e1t_tiles = []
                for u in range(MT):
                    e1tu = e1tpool.tile([128, N], f32, name="E1T", tag="E1T")
                    ps4 = [
                        psb.tile([128, 512], f32, name=f"ps_e1_{k}", tag="psb")
                        for k in range(NQ)
                    ]
                    for j in range(DT):
                        lhsT = qwtd[j][:, u * 128 : (u + 1) * 128]
                        for nq in range(NQ):
                            nc.tensor.matmul(
                                ps4[nq][:],
                                lhsT,
                                ctd[j][:, nq * 512 : (nq + 1) * 512],
                                start=(j == 0),
                                stop=(j == DT - 1),
                            )
                    for nq in range(NQ):
                        nc.scalar.activation(
                            e1tu[:, nq * 512 : (nq + 1) * 512],
                            ps4[nq][:],
                            AF.Exp,
                            bias=q2m_tiles[u][:],
                        )
                    e1t_tiles.append(e1tu)

                # ---- T[u] = (1/colsum2) * sum_n E2[n, m-tile u] * C[n, :]
                t_tiles = []
                for u in range(MT):
                    pst = psb.tile([128, 512], f32, name="ps_T", tag="psb")
                    psc = pss.tile([128, 1], f32, name="ps_cs", tag="pss")
                    for t in range(NT):
                        lhsT = e2_tiles[t][:, u * 128 : (u + 1) * 128]
                        nc.tensor.matmul(
                            pst[:], lhsT, Cn(t)[:],
                            start=(t == 0), stop=(t == NT - 1),
                        )
                        nc.tensor.matmul(
                            psc[:], lhsT, ones[:],
                            start=(t == 0), stop=(t == NT - 1),
                        )
                    r2u = smallpool.tile([128, 1], f32, name="r2", tag="small")
                    nc.vector.reciprocal(r2u[:], psc[:])
                    ttu = tpool.tile([128, 512], f32, name="T", tag="T")
                    nc.scalar.activation(ttu[:], pst[:], AF.Copy, scale=r2u[:])
                    t_tiles.append(ttu)

                # ---- A[t] / Bout[t] = (1/rowsum1) * E1T.T @ {Q, T}
                for g in range(NT // 2):
                    ast = stagepool.tile([128, 2, D], f32, name="Ast", tag="Ast")
                    bst = stagepool.tile([128, 2, D], f32, name="Bst", tag="Bst")
                    for s in range(2):
                        t = g * 2 + s
                        psa = psb.tile([128, 512], f32, name="ps_A", tag="psb")
                        psbb = psb.tile([128, 512], f32, name="ps_B", tag="psb")
                        psr = pss.tile([128, 1], f32, name="ps_rs", tag="pss")
                        for u in range(MT):
                            lhsT = e1t_tiles[u][:, t * 128 : (t + 1) * 128]
                            nc.tensor.matmul(
                                psa[:], lhsT, Qm(u)[:],
                                start=(u == 0), stop=(u == MT - 1),
                            )
                            nc.tensor.matmul(
                                psbb[:], lhsT, t_tiles[u][:],
                                start=(u == 0), stop=(u == MT - 1),
                            )
                            nc.tensor.matmul(
                                psr[:], lhsT, ones[:],
                                start=(u == 0), stop=(u == MT - 1),
                            )
                        r1t = smallpool.tile([128, 1], f32, name="r1", tag="small")
                        nc.vector.reciprocal(r1t[:], psr[:])
                        nc.scalar.activation(
                            ast[:, s, :], psa[:], AF.Copy, scale=r1t[:]
                        )
                        nc.scalar.activation(
                            bst[:, s, :], psbb[:], AF.Copy, scale=r1t[:]
                        )
                    nc.sync.dma_start(
                        A_d[b, g * 256 : (g + 1) * 256, :].rearrange(
                            "(s p) d -> p s d", p=128
                        ),
                        ast[:],
                    )
                    nc.sync.dma_start(
                        Bo_d[b, g * 256 : (g + 1) * 256, :].rearrange(
                            "(s p) d -> p s d", p=128
                        ),
                        bst[:],
                    )

    return nc


_NC = None


def _get_nc():
    global _NC
    if _NC is None:
        _NC = build_nc()
        _NC.finalize()
    return _NC


def _make_in_maps(C, Q, Cmask, Qmask, w):
    C = np.asarray(C, dtype=np.float32)
    Q = np.asarray(Q, dtype=np.float32)
    w = np.asarray(w, dtype=np.float32)
    w1, w2, w3 = w[:D], w[D : 2 * D], w[2 * D :]
    w1r = np.ascontiguousarray(w1.reshape(DT, 128).T)
    w2r = np.ascontiguousarray(w2.reshape(DT, 128).T)
    w3r = np.ascontiguousarray(w3.reshape(DT, 128).T)
    ident = np.eye(128, dtype=np.float32)
    cmb_full = np.asarray(Cmask, dtype=np.float32) * np.float32(NEG)  # [B, N]
    qmb_full = np.asarray(Qmask, dtype=np.float32) * np.float32(NEG)  # [B, M]

    in_maps = []
    for c in range(NCORES):
        bs = slice(c * BPC, (c + 1) * BPC)
        cmb = np.ascontiguousarray(
            cmb_full[bs].reshape(BPC, NT, 128).transpose(2, 0, 1)
        )
        qmb = np.ascontiguousarray(
            qmb_full[bs].reshape(BPC, MT, 128).transpose(2, 0, 1)
        )
        in_maps.append(
            {
                "C": np.ascontiguousarray(C[bs]),
                "Q": np.ascontiguousarray(Q[bs]),
                "cmb": cmb,
                "qmb": qmb,
                "w1r": w1r,
                "w2r": w2r,
                "w3r": w3r,
                "ident": ident,
            }
        )
    return in_maps


def run_spmd(C, Q, Cmask, Qmask, w, trace=False):
    """Returns ((A, Bout), BassKernelResults)."""
    from concourse.bass_utils import run_bass_kernel_spmd

    nc = _get_nc()
    in_maps = _make_in_maps(C, Q, Cmask, Qmask, w)
    res = run_bass_kernel_spmd(nc, in_maps, list(range(NCORES)), trace=trace)
    A = np.concatenate([np.asarray(r["A"]) for r in res.results], axis=0)
    Bout = np.concatenate([np.asarray(r["Bout"]) for r in res.results], axis=0)
    return (A, Bout), res


def kernel(C, Q, Cmask, Qmask, w):
    (A, Bout), _ = run_spmd(
        C, Q, Cmask, Qmask, w, trace=bool(int(os.environ.get("BASSK_TRACE", "0")))
    )
    return (A, Bout)
